# revision 43
# baseline (speedup 1.0000x reference)
"""Multi-head attention on 8 TRN2 NeuronCores (v2).

Problem: x[2, 2048, 1024], w_qkv[1024, 3072], w_out[1024, 1024] (f32).
  qkv = x @ w_qkv; q,k,v per 16 heads of dim 64; softmax(q k^T / 8) v; out proj.

Sharding: 16 heads split 8 ways (one head-PAIR per core, both batches on
every core).  Each core computes q^T/k^T/v for its 2 heads over all
B*L = 4096 rows, runs attention, then an 8-rank AllToAll exchanges
(head-pair -> (batch, L/4-chunk)) so each core finishes the output
projection for its own 512 output rows with all 16 heads present.

v2 changes vs v1:
  * Scores matmuls contract over the 64-dim head axis; v1 zero-padded
    K to 128 (half the PE array wasted).  v2 packs the TWO BATCHES of
    one head into the two K-halves of the array via PE row tiling
    (tile_position (0,0) / (64,0)) so both batches' score tiles compute
    CONCURRENTLY.  q^T/k^T are stored per head with batch-1's columns
    duplicated into partitions 64:128 (sbuf->sbuf DMA) to feed the
    second row group.
  * exp() runs at FD=3072 out of a 6-bank PSUM score tile (one
    ScalarE instruction covers 3 m-tiles x both batches), cutting
    ScalarE per-instruction overhead.
  * Softmax normalization moved AFTER the AllToAll: the exchange
    carries unnormalized o^T plus the per-(l,head) denominators
    (65th row).  The receiver reciprocates denominators via two small
    PE transpose passes, expands them to [128, 512] with one tiny
    selection-matrix matmul per head-pair, and applies one
    tensor-tensor multiply per pair -- replacing v1's 128 PE
    transposes + per-tile DVE normalization.
  * attn@v matmuls of block N are interleaved into the score-fill
    gaps of block N+1 so the PE keeps running while ScalarE exps.
  * Output projection reads w_out and the gathered heads from the
    (dead) xT SBUF slots; psum drains alternate Vector/Scalar copies.

Compute dtype bf16 (f32 accumulation in PSUM).
"""

import sys
import types

sys.path.insert(0, "/opt/trn_rl_repo")

import numpy as np
import ml_dtypes

import concourse.bass as bass
import concourse.mybir as mybir
import concourse.tile as tile
from concourse import bacc
from concourse import bass_utils
from concourse.masks import make_identity

# If the image's antenv lacks the axon_hooks module, run_bass_kernel_spmd's
# trace path (reachable via BASS_TRACE=1) would die on import.  Provide the
# registry so tracing degrades gracefully instead (hook stays None unless
# trn_boot registered one).
try:
    import antenv.axon_hooks  # noqa: F401
except ImportError:
    _hooks = types.ModuleType("antenv.axon_hooks")
    _hooks._hook = None
    _hooks.set_axon_ntff_profile_hook = (
        lambda h: setattr(_hooks, "_hook", h))
    _hooks.get_axon_ntff_profile_hook = lambda: _hooks._hook
    sys.modules["antenv.axon_hooks"] = _hooks

# Artifact upload needs bucket credentials; fall back to the local dir so a
# traced run in a sandboxed container still completes.
_orig_upload = bass_utils.upload_artifacts


def _safe_upload(tmpdir):
    try:
        return _orig_upload(tmpdir)
    except Exception:
        return tmpdir


bass_utils.upload_artifacts = _safe_upload

B, L, D, H, DH = 2, 2048, 1024, 16, 64
BL = B * L  # 4096
SCALE = DH ** -0.5
N_CORES = 8
BF16 = mybir.dt.bfloat16
F32 = mybir.dt.float32
Exp = mybir.ActivationFunctionType.Exp
Mult = mybir.AluOpType.mult

KT = D // 128          # 8 k-tiles over the model dim
MT = L // 128          # 16 m-tiles per batch
LC = L // 512          # 4 l-chunks of 512 per batch
VT = BL // 128         # 32 v row-tiles over (b, l)


def _build():
    nc = bacc.Bacc("TRN2", target_bir_lowering=False, debug=False,
                   num_devices=N_CORES)
    xT_ext = nc.declare_dram_parameter("xT", [D, BL], BF16, isOutput=False)
    wqk_ext = nc.declare_dram_parameter("wqk", [D, 256], BF16, isOutput=False)
    wv_ext = nc.declare_dram_parameter("wv", [D, 128], BF16, isOutput=False)
    wout_ext = nc.declare_dram_parameter("wout", [D, D], BF16, isOutput=False)
    sel_ext = nc.declare_dram_parameter("sel", [64, 8 * 128], BF16,
                                        isOutput=False)
    out_ext = nc.declare_dram_parameter("out", [512, D], F32, isOutput=True)

    with tile.TileContext(nc) as tc:
        with (
            tc.tile_pool(name="big", bufs=1) as big,
            tc.tile_pool(name="pt", bufs=2) as ptp,
            tc.tile_pool(name="psum_st", bufs=2, space="PSUM") as pst,
            tc.tile_pool(name="psum_ov", bufs=2, space="PSUM") as pov,
            tc.tile_pool(name="small", bufs=2) as small,
            tc.tile_pool(name="dram", bufs=1, space="DRAM") as dram,
        ):
            # Input DMAs round-robin over three engines' DMA queues -- a
            # single queue caps the 8MB xT load at ~110 GB/s.
            _dq = [nc.sync, nc.scalar, nc.gpsimd]
            _dqi = [0]

            def dma_in(out, in_):
                _dq[_dqi[0] % 3].dma_start(out, in_)
                _dqi[0] += 1

            # ---- static SBUF tensors ----
            xT_t = [big.tile([128, BL], BF16, tag=f"xT{k}", name=f"xT{k}")
                    for k in range(KT)]
            wqk_t = [big.tile([128, 256], BF16, tag=f"wqk{k}", name=f"wqk{k}")
                     for k in range(KT)]
            wv_t = [big.tile([128, 128], BF16, tag=f"wv{k}", name=f"wv{k}")
                    for k in range(KT)]
            for k in range(KT):
                dma_in(xT_t[k][:, 0:512], xT_ext[k * 128:(k + 1) * 128, 0:512])
                dma_in(wqk_t[k][:], wqk_ext[k * 128:(k + 1) * 128, :])
                dma_in(wv_t[k][:], wv_ext[k * 128:(k + 1) * 128, :])
            for cc in range(1, 8):
                for k in range(KT):
                    dma_in(
                        xT_t[k][:, cc * 512:(cc + 1) * 512],
                        xT_ext[k * 128:(k + 1) * 128, cc * 512:(cc + 1) * 512])

            ident_b = big.tile([128, 128], BF16, tag="ident_b")
            make_identity(nc, ident_b[:])
            ident_f = big.tile([128, 128], F32, tag="ident_f")
            make_identity(nc, ident_f[:])

            # selection matrix for the denominator broadcast: den_all rows
            # are (hl*8 + pair); sel[:, k*128:(k+1)*128] is [16, 128] with
            # row k ones on cols 0:64 (head 0 of pair k) and row 8+k ones
            # on cols 64:128 (head 1).  Host-provided constant.
            sel = big.tile([64, 8 * 128], BF16, tag="sel")
            dma_in(sel[:], sel_ext[:, :])

            # Warm the HAM clock gate during the initial xT DMA wait.
            warm = pst.tile([128, 1536], F32, tag="st", name="warm")
            for i in range(35):
                nc.tensor.matmul(warm[:, 0:128], ident_b[:], ident_b[:],
                                 start=(i == 0), stop=(i == 34))

            # q^T / k^T per head: rows 0:64 = head dims (all 4096 cols),
            # rows 64:128 = duplicate of batch-1's columns so batch-0 and
            # batch-1 score matmuls run in the two PE row-groups at once.
            qq = [[big.tile([128, BL], BF16, tag=f"qp{m}{h}", name=f"qp{m}{h}")
                   for h in range(2)] for m in range(2)]
            # v: per head h, cols [h*65 : h*65+64] = dims, +64 = ones
            v_t = [big.tile([128, 2, 65], BF16, tag=f"v{t}", name=f"v{t}")
                   for t in range(VT)]
            # unnormalized o^T per local head: rows 0:64 = dims, row 64 = den
            oT_h = [big.tile([65, BL], BF16, tag=f"oT{hl}", name=f"oT{hl}")
                    for hl in range(2)]

            # ---- QKV projection (q/k now; v woven into attention) ----
            for ncol in range(8):
                for m in range(2):  # 0 -> q, 1 -> k
                    ps = pov.tile([128, 512], F32, tag="ov",
                                  name=f"qk_ps{ncol}_{m}")
                    for k in range(KT):
                        nc.tensor.matmul(
                            ps[:],
                            wqk_t[k][:, m * 128:(m + 1) * 128],
                            xT_t[k][:, ncol * 512:(ncol + 1) * 512],
                            start=(k == 0), stop=(k == KT - 1),
                        )
                    for h in range(2):
                        nc.vector.tensor_copy(
                            qq[m][h][0:64, ncol * 512:(ncol + 1) * 512],
                            ps[h * 64:(h + 1) * 64, :])
                        if ncol >= 4:
                            # batch-1 window: duplicate into rows 64:128
                            # right away so the first score fills aren't
                            # gated on one big late copy
                            dma_in(qq[m][h][64:128,
                                            ncol * 512:(ncol + 1) * 512],
                                   qq[m][h][0:64,
                                            ncol * 512:(ncol + 1) * 512])

            def v_chain(t):
                ps = pov.tile([128, 128], F32, tag="ov", name=f"v_ps{t}")
                for k in range(KT):
                    nc.tensor.matmul(
                        ps[:],
                        xT_t[k][:, t * 128:(t + 1) * 128],
                        wv_t[k][:],
                        start=(k == 0), stop=(k == KT - 1),
                    )
                nc.vector.tensor_copy(
                    v_t[t][:, :, 0:64],
                    ps[:].rearrange("p (h c) -> p h c", h=2))
                nc.gpsimd.memset(v_t[t][:, :, 64:65], 1.0)

            # ---- collective staging ----
            cc_in = [dram.tile([N_CORES, 65, 512], BF16, name=f"cc_in{i}")
                     for i in range(2)]
            cc_out = [dram.tile([N_CORES, 65, 512], BF16, name=f"cc_out{i}")
                      for i in range(2)]
            # gathered heads (cols 0:512) + w_out (cols 512:1536), aliased
            # onto the dead xT slots
            ogw = [big.tile([128, 1536], BF16, tag=f"xT{k}", name=f"ogw{k}")
                   for k in range(KT)]
            den_h = [big.tile([8, 512], BF16, tag=f"den{hl}",
                              name=f"den{hl}") for hl in range(2)]
            rcpT = small.tile([128, 64], F32, tag="rcpT")
            # reciprocal rows live at 32-aligned bases: hl0 -> rows 0:8,
            # hl1 -> rows 32:40 (engine ops need 32-aligned partition base);
            # the sel matmul contracts over K=64 with zeros elsewhere.
            rcp_sb = small.tile([64, 512], BF16, tag="rcp_sb")
            nc.gpsimd.memset(rcp_sb[:], 0.0)

            def emit_den_path(hl):
                # den_all rows hl*8:(hl+1)*8 -> reciprocals -> rcp_sb rows
                rs = slice(hl * 8, (hl + 1) * 8)
                denT = pov.tile([128, 32], BF16, tag="ov",
                                name=f"denT{hl}")
                for j in range(4):
                    nc.tensor.transpose(denT[:, j * 8:(j + 1) * 8],
                                        den_h[hl][:, j * 128:(j + 1) * 128],
                                        ident_b[0:8, 0:8])
                cs = slice(hl * 32, (hl + 1) * 32)
                nc.vector.reciprocal(rcpT[:, cs], denT[:])
                rcp_ps = pov.tile([8, 512], F32, tag="ov",
                                  name=f"rcp_ps{hl}")
                for j in range(4):
                    nc.tensor.transpose(
                        rcp_ps[:, j * 128:(j + 1) * 128],
                        rcpT[:, hl * 32 + j * 8:hl * 32 + (j + 1) * 8],
                        ident_f[:])
                nc.vector.tensor_copy(rcp_sb[hl * 32:hl * 32 + 8, :],
                                      rcp_ps[:])

            def stage_a2a(hl, wins):
                for w in wins:
                    nc.sync.dma_start(cc_in[hl][w],
                                      oT_h[hl][:, w * 512:(w + 1) * 512])

            def emit_a2a(hl):
                nc.gpsimd.collective_compute(
                    "AllToAll",
                    mybir.AluOpType.bypass,
                    ins=[cc_in[hl].opt()],
                    outs=[cc_out[hl].opt()],
                    replica_groups=[list(range(N_CORES))],
                )
                nc.sync.dma_start(den_h[hl][:], cc_out[hl][:, 64, :])
                for k in range(N_CORES):
                    eng = nc.sync if k % 2 == 0 else nc.gpsimd
                    eng.dma_start(ogw[k][hl * 64:(hl + 1) * 64, 0:512],
                                  cc_out[hl][k, 0:64, :])

            # ---- attention: hl outer, lc inner, batches packed ----
            # Per block (hl, lc): 11 score fills (5 per batch of 3 m-tiles
            # + 1 combined final) into double-buffered 3-bank psum tiles;
            # each fill exp'd by one ScalarE instruction into pt.  The
            # b0/b1 fills use disjoint PE row groups so adjacent fills
            # stream concurrently, and the double buffer keeps ScalarE
            # 100% busy.  attn@v of the PREVIOUS block plus leftover v
            # chains are woven between fills to keep the PE fed.
            from collections import deque
            attnv_q = deque()
            extra_q = deque()
            for t0 in range(0, VT, 2):
                extra_q.append(lambda t0=t0: (v_chain(t0), v_chain(t0 + 1)))

            def attnv_chunks(hl, lc, pt):
                ovs = {}

                def chunk(f):
                    b = f // 3
                    ph = f % 3
                    if ph == 0:
                        ovs[b] = pov.tile([128, 512], F32, tag="ov",
                                          name=f"ov{hl}{lc}{b}")
                    for mt in range(6 * ph, min(6 * ph + 6, MT)):
                        nc.tensor.matmul(
                            ovs[b][0:65, :],
                            v_t[b * MT + mt][:, hl, :],
                            pt[:, b, mt, :],
                            start=(mt == 0), stop=(mt == MT - 1),
                        )
                    if ph == 2:
                        nc.vector.tensor_copy(
                            oT_h[hl][:, (b * LC + lc) * 512:
                                     (b * LC + lc + 1) * 512],
                            ovs[b][0:65, :])
                return [lambda f=f: chunk(f) for f in range(6)]

            def weave():
                # one attn@v chunk per slot when pending, else drain the
                # extra queue (v chains) twice as fast
                if attnv_q:
                    attnv_q.popleft()()
                    if extra_q:
                        extra_q.popleft()()
                else:
                    for _ in range(2):
                        if extra_q:
                            extra_q.popleft()()

            def score_mm(st_ap, hl, lc, b, mt):
                rb = slice(64 * b, 64 * b + 64)
                nc.tensor.matmul(
                    st_ap,
                    qq[1][hl][rb, b * L + mt * 128:b * L + (mt + 1) * 128],
                    qq[0][hl][rb, b * L + lc * 512:b * L + (lc + 1) * 512],
                    start=True, stop=True,
                )

            for hl in range(2):
                for lc in range(4):
                    pt = ptp.tile([128, 2, MT, 512], BF16, tag="pt",
                                  name=f"pt{hl}{lc}")
                    for f in range(5):
                        # paired fills: st_a (b0, PE rows 0:63) and st_b
                        # (b1, rows 64:127) stream concurrently; their two
                        # exps keep ScalarE busy while the next pair fills
                        # the other pool generation.
                        mt0 = 3 * f
                        st_a = pst.tile([128, 1536], F32, tag="st",
                                        name=f"sta{hl}{lc}{f}")
                        st_b = pst.tile([128, 1536], F32, tag="st",
                                        name=f"stb{hl}{lc}{f}")
                        for i in range(3):
                            score_mm(st_a[:, i * 512:(i + 1) * 512],
                                     hl, lc, 0, mt0 + i)
                            score_mm(st_b[:, i * 512:(i + 1) * 512],
                                     hl, lc, 1, mt0 + i)
                        nc.scalar.activation(
                            pt[:, 0, mt0:mt0 + 3, :],
                            st_a[:].rearrange("p (m x) -> p m x", m=3),
                            Exp, scale=SCALE)
                        nc.scalar.activation(
                            pt[:, 1, mt0:mt0 + 3, :],
                            st_b[:].rearrange("p (m x) -> p m x", m=3),
                            Exp, scale=SCALE)
                        weave()
                    # final m-tile, both batches in one tile / one exp
                    st = pst.tile([128, 1024], F32, tag="st",
                                  name=f"stf{hl}{lc}")
                    score_mm(st[:, 0:512], hl, lc, 0, 15)
                    score_mm(st[:, 512:1024], hl, lc, 1, 15)
                    nc.scalar.activation(
                        pt[:, :, 15:16, :],
                        st[:].rearrange("p (b m x) -> p b m x", b=2, m=1),
                        Exp, scale=SCALE)
                    weave()
                    attnv_q.extend(attnv_chunks(hl, lc, pt))
                    if hl == 1 and lc == 0:
                        # (0,3)'s attn@v drained during this block's weave,
                        # so oT_h[0] is complete: overlap the first exchange
                        # with the rest of hl=1.
                        stage_a2a(0, range(N_CORES))
                        emit_a2a(0)

                if hl == 0:
                    # xT is dead once the v chains drained; start loading
                    # w_out into the spare ogw halves under hl=1 compute.
                    for k in range(KT):
                        nc.sync.dma_start(ogw[k][:, 512:1536],
                                          wout_ext[k * 128:(k + 1) * 128, :])
            # hl0 dens landed long ago; their PE transposes go here so
            # they never block younger fills in the in-order PE stream.
            stage_a2a(1, (0, 1, 2, 4, 5, 6))
            emit_den_path(0)
            while attnv_q:
                attnv_q.popleft()()
            stage_a2a(1, (3, 7))
            emit_a2a(1)
            # Keep the PE's HAM clock gate warm through the exchange wait:
            # paced dummy matmuls (each WAR-gated on a DVE copy two
            # generations back) put one short matmul in every ~1us window,
            # so the denominator path / broadcast / output projection all
            # start at full clock instead of K=4/8 half rate.
            for i in range(30):
                wps = pov.tile([128, 512], F32, tag="ov", name=f"wm{i}")
                nc.tensor.matmul(wps[:, 0:128], ident_b[:], ident_b[:],
                                 start=True, stop=True)
                wsb = small.tile([128, 512], F32, tag="osb",
                                 name=f"wmc{i}")
                nc.vector.tensor_copy(wsb[:, 0:128], wps[:, 0:128])

            # ---- denominator broadcast + normalize (rcp_sb was filled
            # per-hl by emit_den_path; hl=0 ran under hl=1 compute) ----
            emit_den_path(1)
            # normalized gathered heads, aliased onto the dead q^T slot
            ogn = big.tile([128, 8 * 512], BF16, tag="qp00", name="ogn")
            for g in range(3):  # 3-bank bcast generations
                ks = range(3 * g, min(3 * g + 3, 8))
                bc = pst.tile([128, 1536], F32, tag="st", name=f"bcast{g}")
                for i, k in enumerate(ks):
                    nc.tensor.matmul(
                        bc[:, i * 512:(i + 1) * 512],
                        sel[:, k * 128:(k + 1) * 128],
                        rcp_sb[:],
                        start=True, stop=True,
                    )
                for i, k in enumerate(ks):
                    nc.vector.scalar_tensor_tensor(
                        ogn[:, k * 512:(k + 1) * 512],
                        ogw[k][:, 0:512], 1.0,
                        bc[:, i * 512:(i + 1) * 512],
                        Mult, Mult)

            # ---- output projection for our 512 rows ----
            for lt in range(4):
                for nt in range(2):
                    ps = pov.tile([128, 512], F32, tag="ov",
                                  name=f"op{lt}{nt}")
                    for k in range(KT):
                        nc.tensor.matmul(
                            ps[:],
                            ogn[:, k * 512 + lt * 128:k * 512 + (lt + 1) * 128],
                            ogw[k][:, 512 + nt * 512:512 + (nt + 1) * 512],
                            start=(k == 0), stop=(k == KT - 1),
                        )
                    osb = small.tile([128, 512], F32, tag="osb")
                    if nt == 0:
                        nc.vector.tensor_copy(osb[:], ps[:])
                    else:
                        nc.scalar.copy(osb[:], ps[:])
                    (nc.sync if nt == 0 else nc.scalar).dma_start(
                        out_ext[lt * 128:(lt + 1) * 128,
                                nt * 512:(nt + 1) * 512],
                        osb[:])

    nc.compile()
    return nc


_NC_CACHE = None


def _get_nc():
    global _NC_CACHE
    if _NC_CACHE is None:
        _NC_CACHE = _build()
    return _NC_CACHE


def _make_in_maps(x, w_qkv, w_out):
    x = np.asarray(x, dtype=np.float32)
    w_qkv = np.asarray(w_qkv, dtype=np.float32)
    w_out = np.asarray(w_out, dtype=np.float32)
    bf = ml_dtypes.bfloat16
    xT = np.ascontiguousarray(
        x.transpose(2, 0, 1).reshape(D, BL)).astype(bf)
    wout_b = w_out.astype(bf)
    sel = np.zeros((64, 8 * 128), dtype=np.float32)
    for k in range(N_CORES):
        sel[k, k * 128:k * 128 + 64] = 1.0
        sel[32 + k, k * 128 + 64:(k + 1) * 128] = 1.0
    sel = sel.astype(bf)
    in_maps = []
    for c in range(N_CORES):
        cs = slice(c * 128, (c + 1) * 128)
        wqk_c = np.ascontiguousarray(
            np.concatenate([w_qkv[:, cs], w_qkv[:, D:][:, cs]], axis=1)
        ).astype(bf)
        wv_c = np.ascontiguousarray(w_qkv[:, 2 * D:][:, cs]).astype(bf)
        in_maps.append({"xT": xT, "wqk": wqk_c, "wv": wv_c, "wout": wout_b,
                        "sel": sel})
    return in_maps


def _run(x, w_qkv, w_out, trace=False):
    nc = _get_nc()
    in_maps = _make_in_maps(x, w_qkv, w_out)
    res = bass_utils.run_bass_kernel_spmd(
        nc, in_maps, list(range(N_CORES)), trace=trace)
    out = np.empty((B, L, D), dtype=np.float32)
    for c in range(N_CORES):
        out[c // 4, (c % 4) * 512:(c % 4 + 1) * 512, :] = \
            np.asarray(res.results[c]["out"])
    return out, res


def kernel(x, w_qkv, w_out):
    out, _ = _run(x, w_qkv, w_out, trace=False)
    return out


# revision 44
# speedup vs baseline: 1.0396x; 1.0396x over previous
"""Multi-head attention on 8 TRN2 NeuronCores (v2).

Problem: x[2, 2048, 1024], w_qkv[1024, 3072], w_out[1024, 1024] (f32).
  qkv = x @ w_qkv; q,k,v per 16 heads of dim 64; softmax(q k^T / 8) v; out proj.

Sharding: 16 heads split 8 ways (one head-PAIR per core, both batches on
every core).  Each core computes q^T/k^T/v for its 2 heads over all
B*L = 4096 rows, runs attention, then an 8-rank AllToAll exchanges
(head-pair -> (batch, L/4-chunk)) so each core finishes the output
projection for its own 512 output rows with all 16 heads present.

v2 changes vs v1:
  * Scores matmuls contract over the 64-dim head axis; v1 zero-padded
    K to 128 (half the PE array wasted).  v2 packs the TWO BATCHES of
    one head into the two K-halves of the array via PE row tiling
    (tile_position (0,0) / (64,0)) so both batches' score tiles compute
    CONCURRENTLY.  q^T/k^T are stored per head with batch-1's columns
    duplicated into partitions 64:128 (sbuf->sbuf DMA) to feed the
    second row group.
  * exp() runs at FD=3072 out of a 6-bank PSUM score tile (one
    ScalarE instruction covers 3 m-tiles x both batches), cutting
    ScalarE per-instruction overhead.
  * Softmax normalization moved AFTER the AllToAll: the exchange
    carries unnormalized o^T plus the per-(l,head) denominators
    (65th row).  The receiver reciprocates denominators via two small
    PE transpose passes, expands them to [128, 512] with one tiny
    selection-matrix matmul per head-pair, and applies one
    tensor-tensor multiply per pair -- replacing v1's 128 PE
    transposes + per-tile DVE normalization.
  * attn@v matmuls of block N are interleaved into the score-fill
    gaps of block N+1 so the PE keeps running while ScalarE exps.
  * Output projection reads w_out and the gathered heads from the
    (dead) xT SBUF slots; psum drains alternate Vector/Scalar copies.

Compute dtype bf16 (f32 accumulation in PSUM).
"""

import sys
import types

sys.path.insert(0, "/opt/trn_rl_repo")

import numpy as np
import ml_dtypes

import concourse.bass as bass
import concourse.mybir as mybir
import concourse.tile as tile
from concourse import bacc
from concourse import bass_utils
from concourse.masks import make_identity

# If the image's antenv lacks the axon_hooks module, run_bass_kernel_spmd's
# trace path (reachable via BASS_TRACE=1) would die on import.  Provide the
# registry so tracing degrades gracefully instead (hook stays None unless
# trn_boot registered one).
try:
    import antenv.axon_hooks  # noqa: F401
except ImportError:
    _hooks = types.ModuleType("antenv.axon_hooks")
    _hooks._hook = None
    _hooks.set_axon_ntff_profile_hook = (
        lambda h: setattr(_hooks, "_hook", h))
    _hooks.get_axon_ntff_profile_hook = lambda: _hooks._hook
    sys.modules["antenv.axon_hooks"] = _hooks

# Artifact upload needs bucket credentials; fall back to the local dir so a
# traced run in a sandboxed container still completes.
_orig_upload = bass_utils.upload_artifacts


def _safe_upload(tmpdir):
    try:
        return _orig_upload(tmpdir)
    except Exception:
        return tmpdir


bass_utils.upload_artifacts = _safe_upload

B, L, D, H, DH = 2, 2048, 1024, 16, 64
BL = B * L  # 4096
SCALE = DH ** -0.5
N_CORES = 8
BF16 = mybir.dt.bfloat16
F32 = mybir.dt.float32
Exp = mybir.ActivationFunctionType.Exp
Mult = mybir.AluOpType.mult

KT = D // 128          # 8 k-tiles over the model dim
MT = L // 128          # 16 m-tiles per batch
LC = L // 512          # 4 l-chunks of 512 per batch
VT = BL // 128         # 32 v row-tiles over (b, l)


def _build():
    nc = bacc.Bacc("TRN2", target_bir_lowering=False, debug=False,
                   num_devices=N_CORES)
    xT_ext = nc.declare_dram_parameter("xT", [D, BL], BF16, isOutput=False)
    wqk_ext = nc.declare_dram_parameter("wqk", [D, 256], BF16, isOutput=False)
    wv_ext = nc.declare_dram_parameter("wv", [D, 128], BF16, isOutput=False)
    wout_ext = nc.declare_dram_parameter("wout", [D, D], BF16, isOutput=False)
    sel_ext = nc.declare_dram_parameter("sel", [64, 8 * 128], BF16,
                                        isOutput=False)
    out_ext = nc.declare_dram_parameter("out", [512, D], F32, isOutput=True)

    with tile.TileContext(nc) as tc:
        with (
            tc.tile_pool(name="big", bufs=1) as big,
            tc.tile_pool(name="pt", bufs=2) as ptp,
            tc.tile_pool(name="psum_st", bufs=2, space="PSUM") as pst,
            tc.tile_pool(name="psum_ov", bufs=2, space="PSUM") as pov,
            tc.tile_pool(name="small", bufs=2) as small,
            tc.tile_pool(name="dram", bufs=1, space="DRAM") as dram,
        ):
            # Input DMAs round-robin over three engines' DMA queues -- a
            # single queue caps the 8MB xT load at ~110 GB/s.
            _dq = [nc.sync, nc.scalar, nc.gpsimd]
            _dqi = [0]

            def dma_in(out, in_):
                _dq[_dqi[0] % 3].dma_start(out, in_)
                _dqi[0] += 1

            # ---- static SBUF tensors ----
            xT_t = [big.tile([128, BL], BF16, tag=f"xT{k}", name=f"xT{k}")
                    for k in range(KT)]
            wqk_t = [big.tile([128, 256], BF16, tag=f"wqk{k}", name=f"wqk{k}")
                     for k in range(KT)]
            wv_t = [big.tile([128, 128], BF16, tag=f"wv{k}", name=f"wv{k}")
                    for k in range(KT)]
            for k in range(KT):
                dma_in(xT_t[k][:, 0:512], xT_ext[k * 128:(k + 1) * 128, 0:512])
                dma_in(wqk_t[k][:], wqk_ext[k * 128:(k + 1) * 128, :])
                dma_in(wv_t[k][:], wv_ext[k * 128:(k + 1) * 128, :])
            for cc in range(1, 8):
                for k in range(KT):
                    dma_in(
                        xT_t[k][:, cc * 512:(cc + 1) * 512],
                        xT_ext[k * 128:(k + 1) * 128, cc * 512:(cc + 1) * 512])

            ident_b = big.tile([128, 128], BF16, tag="ident_b")
            make_identity(nc, ident_b[:])
            ident_f = big.tile([128, 128], F32, tag="ident_f")
            make_identity(nc, ident_f[:])

            # selection matrix for the denominator broadcast: den_all rows
            # are (hl*8 + pair); sel[:, k*128:(k+1)*128] is [16, 128] with
            # row k ones on cols 0:64 (head 0 of pair k) and row 8+k ones
            # on cols 64:128 (head 1).  Host-provided constant.
            sel = big.tile([64, 8 * 128], BF16, tag="sel")
            dma_in(sel[:], sel_ext[:, :])

            # Warm the HAM clock gate during the initial xT DMA wait.
            warm = pst.tile([128, 1536], F32, tag="st", name="warm")
            for i in range(35):
                nc.tensor.matmul(warm[:, 0:128], ident_b[:], ident_b[:],
                                 start=(i == 0), stop=(i == 34))

            # q^T / k^T per head: rows 0:64 = head dims (all 4096 cols),
            # rows 64:128 = duplicate of batch-1's columns so batch-0 and
            # batch-1 score matmuls run in the two PE row-groups at once.
            qq = [[big.tile([128, BL], BF16, tag=f"qp{m}{h}", name=f"qp{m}{h}")
                   for h in range(2)] for m in range(2)]
            # v: per head h, cols [h*65 : h*65+64] = dims, +64 = ones
            v_t = [big.tile([128, 2, 65], BF16, tag=f"v{t}", name=f"v{t}")
                   for t in range(VT)]
            # unnormalized o^T per local head: rows 0:64 = dims, row 64 = den
            oT_h = [big.tile([65, BL], BF16, tag=f"oT{hl}", name=f"oT{hl}")
                    for hl in range(2)]

            # ---- QKV projection (q/k now; v woven into attention) ----
            for ncol in range(8):
                for m in range(2):  # 0 -> q, 1 -> k
                    ps = pov.tile([128, 512], F32, tag="ov",
                                  name=f"qk_ps{ncol}_{m}")
                    for k in range(KT):
                        nc.tensor.matmul(
                            ps[:],
                            wqk_t[k][:, m * 128:(m + 1) * 128],
                            xT_t[k][:, ncol * 512:(ncol + 1) * 512],
                            start=(k == 0), stop=(k == KT - 1),
                        )
                    for h in range(2):
                        nc.vector.tensor_copy(
                            qq[m][h][0:64, ncol * 512:(ncol + 1) * 512],
                            ps[h * 64:(h + 1) * 64, :])
                        if ncol >= 4:
                            # batch-1 window: duplicate into rows 64:128
                            # right away so the first score fills aren't
                            # gated on one big late copy
                            dma_in(qq[m][h][64:128,
                                            ncol * 512:(ncol + 1) * 512],
                                   qq[m][h][0:64,
                                            ncol * 512:(ncol + 1) * 512])

            def v_chain(t):
                ps = pov.tile([128, 128], F32, tag="ov", name=f"v_ps{t}")
                for k in range(KT):
                    nc.tensor.matmul(
                        ps[:],
                        xT_t[k][:, t * 128:(t + 1) * 128],
                        wv_t[k][:],
                        start=(k == 0), stop=(k == KT - 1),
                    )
                nc.vector.tensor_copy(
                    v_t[t][:, :, 0:64],
                    ps[:].rearrange("p (h c) -> p h c", h=2))
                nc.gpsimd.memset(v_t[t][:, :, 64:65], 1.0)

            # ---- collective staging ----
            cc_in = [dram.tile([N_CORES, 65, 512], BF16, name=f"cc_in{i}")
                     for i in range(2)]
            cc_out = [dram.tile([N_CORES, 65, 512], BF16, name=f"cc_out{i}")
                      for i in range(2)]
            # gathered heads (cols 0:512) + w_out (cols 512:1536), aliased
            # onto the dead xT slots
            ogw = [big.tile([128, 1536], BF16, tag=f"xT{k}", name=f"ogw{k}")
                   for k in range(KT)]
            den_h = [big.tile([8, 512], BF16, tag=f"den{hl}",
                              name=f"den{hl}") for hl in range(2)]
            rcpT = small.tile([128, 64], F32, tag="rcpT")
            # reciprocal rows live at 32-aligned bases: hl0 -> rows 0:8,
            # hl1 -> rows 32:40 (engine ops need 32-aligned partition base);
            # the sel matmul contracts over K=64 with zeros elsewhere.
            rcp_sb = small.tile([64, 512], BF16, tag="rcp_sb")
            nc.gpsimd.memset(rcp_sb[:], 0.0)

            def emit_den_path(hl):
                # den_all rows hl*8:(hl+1)*8 -> reciprocals -> rcp_sb rows
                rs = slice(hl * 8, (hl + 1) * 8)
                denT = pov.tile([128, 32], BF16, tag="ov",
                                name=f"denT{hl}")
                for j in range(4):
                    nc.tensor.transpose(denT[:, j * 8:(j + 1) * 8],
                                        den_h[hl][:, j * 128:(j + 1) * 128],
                                        ident_b[0:8, 0:8])
                cs = slice(hl * 32, (hl + 1) * 32)
                nc.vector.reciprocal(rcpT[:, cs], denT[:])
                rcp_ps = pov.tile([8, 512], F32, tag="ov",
                                  name=f"rcp_ps{hl}")
                for j in range(4):
                    nc.tensor.transpose(
                        rcp_ps[:, j * 128:(j + 1) * 128],
                        rcpT[:, hl * 32 + j * 8:hl * 32 + (j + 1) * 8],
                        ident_f[:])
                nc.vector.tensor_copy(rcp_sb[hl * 32:hl * 32 + 8, :],
                                      rcp_ps[:])

            def emit_a2a(hl):
                for j in range(N_CORES):
                    nc.sync.dma_start(cc_in[hl][j],
                                      oT_h[hl][:, j * 512:(j + 1) * 512])
                nc.gpsimd.collective_compute(
                    "AllToAll",
                    mybir.AluOpType.bypass,
                    ins=[cc_in[hl].opt()],
                    outs=[cc_out[hl].opt()],
                    replica_groups=[list(range(N_CORES))],
                )
                nc.sync.dma_start(den_h[hl][:], cc_out[hl][:, 64, :])
                for k in range(N_CORES):
                    eng = nc.sync if k % 2 == 0 else nc.gpsimd
                    eng.dma_start(ogw[k][hl * 64:(hl + 1) * 64, 0:512],
                                  cc_out[hl][k, 0:64, :])

            # ---- attention: hl outer, lc inner, batches packed ----
            # Per block (hl, lc): 11 score fills (5 per batch of 3 m-tiles
            # + 1 combined final) into double-buffered 3-bank psum tiles;
            # each fill exp'd by one ScalarE instruction into pt.  The
            # b0/b1 fills use disjoint PE row groups so adjacent fills
            # stream concurrently, and the double buffer keeps ScalarE
            # 100% busy.  attn@v of the PREVIOUS block plus leftover v
            # chains are woven between fills to keep the PE fed.
            from collections import deque
            attnv_q = deque()
            extra_q = deque()
            for t0 in range(0, VT, 2):
                extra_q.append(lambda t0=t0: (v_chain(t0), v_chain(t0 + 1)))

            def attnv_chunks(hl, lc, pt):
                ovs = {}

                def chunk(f):
                    b = f // 3
                    ph = f % 3
                    if ph == 0:
                        ovs[b] = pov.tile([128, 512], F32, tag="ov",
                                          name=f"ov{hl}{lc}{b}")
                    for mt in range(6 * ph, min(6 * ph + 6, MT)):
                        nc.tensor.matmul(
                            ovs[b][0:65, :],
                            v_t[b * MT + mt][:, hl, :],
                            pt[:, b, mt, :],
                            start=(mt == 0), stop=(mt == MT - 1),
                        )
                    if ph == 2:
                        nc.vector.tensor_copy(
                            oT_h[hl][:, (b * LC + lc) * 512:
                                     (b * LC + lc + 1) * 512],
                            ovs[b][0:65, :])
                return [lambda f=f: chunk(f) for f in range(6)]

            def weave():
                # one attn@v chunk per slot when pending, else drain the
                # extra queue (v chains) twice as fast
                if attnv_q:
                    attnv_q.popleft()()
                    if extra_q:
                        extra_q.popleft()()
                else:
                    for _ in range(2):
                        if extra_q:
                            extra_q.popleft()()

            def score_mm(st_ap, hl, lc, b, mt):
                rb = slice(64 * b, 64 * b + 64)
                nc.tensor.matmul(
                    st_ap,
                    qq[1][hl][rb, b * L + mt * 128:b * L + (mt + 1) * 128],
                    qq[0][hl][rb, b * L + lc * 512:b * L + (lc + 1) * 512],
                    start=True, stop=True,
                )

            for hl in range(2):
                for lc in range(4):
                    pt = ptp.tile([128, 2, MT, 512], BF16, tag="pt",
                                  name=f"pt{hl}{lc}")
                    for f in range(5):
                        # paired fills: st_a (b0, PE rows 0:63) and st_b
                        # (b1, rows 64:127) stream concurrently; their two
                        # exps keep ScalarE busy while the next pair fills
                        # the other pool generation.
                        mt0 = 3 * f
                        st_a = pst.tile([128, 1536], F32, tag="st",
                                        name=f"sta{hl}{lc}{f}")
                        st_b = pst.tile([128, 1536], F32, tag="st",
                                        name=f"stb{hl}{lc}{f}")
                        for i in range(3):
                            score_mm(st_a[:, i * 512:(i + 1) * 512],
                                     hl, lc, 0, mt0 + i)
                            score_mm(st_b[:, i * 512:(i + 1) * 512],
                                     hl, lc, 1, mt0 + i)
                        nc.scalar.activation(
                            pt[:, 0, mt0:mt0 + 3, :],
                            st_a[:].rearrange("p (m x) -> p m x", m=3),
                            Exp, scale=SCALE)
                        nc.scalar.activation(
                            pt[:, 1, mt0:mt0 + 3, :],
                            st_b[:].rearrange("p (m x) -> p m x", m=3),
                            Exp, scale=SCALE)
                        weave()
                    # final m-tile, both batches in one tile / one exp
                    st = pst.tile([128, 1024], F32, tag="st",
                                  name=f"stf{hl}{lc}")
                    score_mm(st[:, 0:512], hl, lc, 0, 15)
                    score_mm(st[:, 512:1024], hl, lc, 1, 15)
                    nc.scalar.activation(
                        pt[:, :, 15:16, :],
                        st[:].rearrange("p (b m x) -> p b m x", b=2, m=1),
                        Exp, scale=SCALE)
                    weave()
                    attnv_q.extend(attnv_chunks(hl, lc, pt))
                    if hl == 1 and lc == 0:
                        # (0,3)'s attn@v drained during this block's weave,
                        # so oT_h[0] is complete: overlap the first exchange
                        # with the rest of hl=1.
                        emit_a2a(0)

                if hl == 0:
                    # xT is dead once the v chains drained; start loading
                    # w_out into the spare ogw halves under hl=1 compute.
                    for k in range(KT):
                        nc.sync.dma_start(ogw[k][:, 512:1536],
                                          wout_ext[k * 128:(k + 1) * 128, :])
            # hl0 dens landed long ago; their PE transposes go here so
            # they never block younger fills in the in-order PE stream.
            emit_den_path(0)
            while attnv_q:
                attnv_q.popleft()()
            emit_a2a(1)
            # Keep the PE's HAM clock gate warm through the exchange wait:
            # paced dummy matmuls (each WAR-gated on a DVE copy two
            # generations back) put one short matmul in every ~1us window,
            # so the denominator path / broadcast / output projection all
            # start at full clock instead of K=4/8 half rate.
            for i in range(30):
                wps = pov.tile([128, 512], F32, tag="ov", name=f"wm{i}")
                nc.tensor.matmul(wps[:, 0:128], ident_b[:], ident_b[:],
                                 start=True, stop=True)
                wsb = small.tile([128, 512], F32, tag="osb",
                                 name=f"wmc{i}")
                nc.vector.tensor_copy(wsb[:, 0:128], wps[:, 0:128])

            # ---- denominator broadcast + normalize (rcp_sb was filled
            # per-hl by emit_den_path; hl=0 ran under hl=1 compute) ----
            emit_den_path(1)
            # normalized gathered heads, aliased onto the dead q^T slot
            ogn = big.tile([128, 8 * 512], BF16, tag="qp00", name="ogn")
            for g in range(3):  # 3-bank bcast generations
                ks = range(3 * g, min(3 * g + 3, 8))
                bc = pst.tile([128, 1536], F32, tag="st", name=f"bcast{g}")
                for i, k in enumerate(ks):
                    nc.tensor.matmul(
                        bc[:, i * 512:(i + 1) * 512],
                        sel[:, k * 128:(k + 1) * 128],
                        rcp_sb[:],
                        start=True, stop=True,
                    )
                for i, k in enumerate(ks):
                    nc.vector.scalar_tensor_tensor(
                        ogn[:, k * 512:(k + 1) * 512],
                        ogw[k][:, 0:512], 1.0,
                        bc[:, i * 512:(i + 1) * 512],
                        Mult, Mult)

            # ---- output projection for our 512 rows ----
            for lt in range(4):
                for nt in range(2):
                    ps = pov.tile([128, 512], F32, tag="ov",
                                  name=f"op{lt}{nt}")
                    for k in range(KT):
                        nc.tensor.matmul(
                            ps[:],
                            ogn[:, k * 512 + lt * 128:k * 512 + (lt + 1) * 128],
                            ogw[k][:, 512 + nt * 512:512 + (nt + 1) * 512],
                            start=(k == 0), stop=(k == KT - 1),
                        )
                    osb = small.tile([128, 512], F32, tag="osb")
                    if nt == 0:
                        nc.vector.tensor_copy(osb[:], ps[:])
                    else:
                        nc.scalar.copy(osb[:], ps[:])
                    (nc.sync if nt == 0 else nc.scalar).dma_start(
                        out_ext[lt * 128:(lt + 1) * 128,
                                nt * 512:(nt + 1) * 512],
                        osb[:])

    nc.compile()
    return nc


_NC_CACHE = None


def _get_nc():
    global _NC_CACHE
    if _NC_CACHE is None:
        _NC_CACHE = _build()
    return _NC_CACHE


def _make_in_maps(x, w_qkv, w_out):
    x = np.asarray(x, dtype=np.float32)
    w_qkv = np.asarray(w_qkv, dtype=np.float32)
    w_out = np.asarray(w_out, dtype=np.float32)
    bf = ml_dtypes.bfloat16
    xT = np.ascontiguousarray(
        x.transpose(2, 0, 1).reshape(D, BL)).astype(bf)
    wout_b = w_out.astype(bf)
    sel = np.zeros((64, 8 * 128), dtype=np.float32)
    for k in range(N_CORES):
        sel[k, k * 128:k * 128 + 64] = 1.0
        sel[32 + k, k * 128 + 64:(k + 1) * 128] = 1.0
    sel = sel.astype(bf)
    in_maps = []
    for c in range(N_CORES):
        cs = slice(c * 128, (c + 1) * 128)
        wqk_c = np.ascontiguousarray(
            np.concatenate([w_qkv[:, cs], w_qkv[:, D:][:, cs]], axis=1)
        ).astype(bf)
        wv_c = np.ascontiguousarray(w_qkv[:, 2 * D:][:, cs]).astype(bf)
        in_maps.append({"xT": xT, "wqk": wqk_c, "wv": wv_c, "wout": wout_b,
                        "sel": sel})
    return in_maps


def _run(x, w_qkv, w_out, trace=False):
    nc = _get_nc()
    in_maps = _make_in_maps(x, w_qkv, w_out)
    res = bass_utils.run_bass_kernel_spmd(
        nc, in_maps, list(range(N_CORES)), trace=trace)
    out = np.empty((B, L, D), dtype=np.float32)
    for c in range(N_CORES):
        out[c // 4, (c % 4) * 512:(c % 4 + 1) * 512, :] = \
            np.asarray(res.results[c]["out"])
    return out, res


def kernel(x, w_qkv, w_out):
    out, _ = _run(x, w_qkv, w_out, trace=False)
    return out
